# revision 1
# baseline (speedup 1.0000x reference)
"""Trainium2 Bass kernel for nn_AdversarialDecomposer (skip-gram + party classifier).

Strategy (data-parallel over batch, 8 cores, 4096 rows each):
  - emb = center_emb[center_ids]: indirect DMA gathers, 128 rows/call (the
    HW-supported shape: one index per partition)
  - enc^T via PE (transpose emb tiles on PE, W_enc^T chunks as stationary)
  - center_v = enc @ W_dec^T (PE), rows at ravel position b=128t+p
  - pos/neg scores: 11 streams x 32 gathers of context_emb rows, fused with
    one big DVE multiply + free-dim reduce per stream
  - softplus terms as -ln(sigmoid(+-s)) on ACT with free-dim accumulate
  - per-core partial sums [1,2] -> host combines, means, clips.

Host staging only reorders/pads/casts inputs; all batch-scale FLOPs on device.
"""

import sys

sys.path.insert(0, "/opt/trn_rl_repo")

import numpy as np

VOCAB = 100000
EMBED = 300
HIDDEN = 300
BATCH = 32768
K_NEG = 10
N_CORES = 8
BC = BATCH // N_CORES  # 4096 rows per core
NT = BC // 128  # 32 batch tiles per core
NK = K_NEG + 1  # 11 gather streams: k=0 is the positive context
EPS = 1e-5
PAD = 384  # 300 padded to 3*128 for PE contraction chunks
NCHUNK = PAD // 128

_PROGRAM = None


def _build_program():
    import contextlib

    import concourse.bass as bass
    import concourse.bacc as bacc
    import concourse.tile as tile
    from concourse import mybir
    f32 = mybir.dt.float32
    i32 = mybir.dt.int32

    nc = bacc.Bacc(
        "TRN2", target_bir_lowering=False, debug=False, num_devices=N_CORES
    )

    # ---- DRAM IO ----
    center_tab = nc.dram_tensor("center_tab", [VOCAB, EMBED], f32, kind="ExternalInput")
    ctx_tab = nc.dram_tensor("ctx_tab", [VOCAB, EMBED], f32, kind="ExternalInput")
    # [p, k*NT+t] = id of stream k for batch row b=128t+p (stream 0: center ids)
    pids_d = nc.dram_tensor("pids", [128, (NK + 1) * NT], i32, kind="ExternalInput")
    sy_d = nc.dram_tensor("sy", [128, NT], f32, kind="ExternalInput")
    wencT_d = nc.dram_tensor("wencT", [128, NCHUNK * PAD], f32, kind="ExternalInput")
    wdecT_d = nc.dram_tensor("wdecT", [128, NCHUNK * EMBED], f32, kind="ExternalInput")
    wclsT_d = nc.dram_tensor("wclsT", [128, NCHUNK * 2], f32, kind="ExternalInput")
    benc_d = nc.dram_tensor("benc", [128, NCHUNK], f32, kind="ExternalInput")
    bdec_d = nc.dram_tensor("bdec", [1, EMBED], f32, kind="ExternalInput")
    bcls_d = nc.dram_tensor("bcls", [1, 2], f32, kind="ExternalInput")
    ident_d = nc.dram_tensor("ident", [128, 128], f32, kind="ExternalInput")
    out_d = nc.dram_tensor("out", [1, 2], f32, kind="ExternalOutput")

    with tile.TileContext(nc) as tc:
        with contextlib.ExitStack() as ctx:
            singles = ctx.enter_context(tc.tile_pool(name="singles", bufs=1))
            persist = ctx.enter_context(tc.tile_pool(name="persist", bufs=1))
            gpool = ctx.enter_context(tc.tile_pool(name="gpool", bufs=4))
            work = ctx.enter_context(tc.tile_pool(name="work", bufs=3))
            psA = ctx.enter_context(tc.tile_pool(name="psA", bufs=2, space="PSUM"))
            psB = ctx.enter_context(tc.tile_pool(name="psB", bufs=2, space="PSUM"))
            psC = ctx.enter_context(tc.tile_pool(name="psC", bufs=2, space="PSUM"))
            psD = ctx.enter_context(tc.tile_pool(name="psD", bufs=1, space="PSUM"))

            # ---- load small inputs into SBUF ----
            pids = singles.tile([128, (NK + 1) * NT], i32)
            nc.sync.dma_start(out=pids[:], in_=pids_d.ap())
            sy = singles.tile([128, NT], f32)
            nc.sync.dma_start(out=sy[:], in_=sy_d.ap())
            wencT = singles.tile([128, NCHUNK, PAD], f32)
            nc.sync.dma_start(out=wencT[:], in_=wencT_d.ap())
            wdecT = singles.tile([128, NCHUNK, EMBED], f32)
            nc.sync.dma_start(out=wdecT[:], in_=wdecT_d.ap())
            wclsT = singles.tile([128, NCHUNK, 2], f32)
            nc.sync.dma_start(out=wclsT[:], in_=wclsT_d.ap())
            benc = singles.tile([128, NCHUNK], f32)
            nc.sync.dma_start(out=benc[:], in_=benc_d.ap())
            bdec = singles.tile([128, EMBED], f32)
            bdec_src = bass.AP(
                tensor=bdec_d, offset=0, ap=[[0, 128]] + bdec_d.ap().ap[1:]
            )
            nc.sync.dma_start(out=bdec[:], in_=bdec_src)
            bcls = singles.tile([128, 2], f32)
            bcls_src = bass.AP(
                tensor=bcls_d, offset=0, ap=[[0, 128]] + bcls_d.ap().ap[1:]
            )
            nc.sync.dma_start(out=bcls[:], in_=bcls_src)

            identity = singles.tile([128, 128], f32)
            nc.sync.dma_start(out=identity[:], in_=ident_d.ap())

            ones = singles.tile([128, 1], f32)
            nc.vector.memset(ones[:], 1.0)

            # ---- persistent buffers ----
            # center_v split into two half-tiles so phase-2 half-stream muls
            # unblock as soon as the first 16 batch tiles finish phase 1
            HNT = NT // 2
            cvA = persist.tile([128, HNT, EMBED], f32)
            cvB = persist.tile([128, HNT, EMBED], f32)
            lg = persist.tile([128, NT, 2], f32)  # logits
            deno_cols = persist.tile([128, NK], f32)  # per-k sum ln(sigmoid(+-s))
            scono = persist.tile([128, 2], f32)
            # persistent emb buffer: all 32 gathers issue up-front so the PE
            # transpose/matmul chain never waits on per-tile DMA completion
            emb = persist.tile([128, NT, PAD], f32)
            nc.vector.memset(emb[:, :, EMBED:PAD], 0.0)
            for t in range(NT):
                nc.gpsimd.indirect_dma_start(
                    out=emb[:, t, 0:EMBED],
                    out_offset=None,
                    in_=center_tab.ap(),
                    in_offset=bass.IndirectOffsetOnAxis(ap=pids[:, t : t + 1], axis=0),
                )

            # ---- phase 1: per tile t: transpose, enc^T, cv, lg ----
            for t in range(NT):
                embT = work.tile([128, NCHUNK, 128], f32, tag="embT")
                for e in range(NCHUNK):
                    tp = psA.tile([128, 128], f32, tag="tp")
                    nc.tensor.transpose(
                        out=tp[:],
                        in_=emb[:, t, 128 * e : 128 * (e + 1)],
                        identity=identity[:],
                    )
                    nc.scalar.activation(
                        out=embT[:, e, :],
                        in_=tp[:],
                        func=mybir.ActivationFunctionType.Copy,
                    )
                encT = work.tile([128, NCHUNK, 128], f32, tag="encT")
                for hc in range(NCHUNK):
                    encP = psB.tile([128, 128], f32, tag="encP")
                    for e in range(NCHUNK):
                        nc.tensor.matmul(
                            out=encP[:],
                            lhsT=wencT[:, e, 128 * hc : 128 * (hc + 1)],
                            rhs=embT[:, e, :],
                            start=(e == 0),
                            stop=(e == NCHUNK - 1),
                        )
                    nc.vector.tensor_tensor(
                        out=encT[:, hc, :],
                        in0=encP[:],
                        in1=benc[:, hc : hc + 1].to_broadcast([128, 128]),
                        op=mybir.AluOpType.add,
                    )
                cvP = psC.tile([128, EMBED], f32, tag="cvP")
                for hc in range(NCHUNK):
                    nc.tensor.matmul(
                        out=cvP[:],
                        lhsT=encT[:, hc, :],
                        rhs=wdecT[:, hc, :],
                        start=(hc == 0),
                        stop=(hc == NCHUNK - 1),
                    )
                cv_half = cvA if t < HNT else cvB
                nc.vector.tensor_tensor(
                    out=cv_half[:, t % HNT, :],
                    in0=cvP[:],
                    in1=bdec[:, :],
                    op=mybir.AluOpType.add,
                )
                lgP = psD.tile([128, 2], f32, tag="lgP")
                for hc in range(NCHUNK):
                    nc.tensor.matmul(
                        out=lgP[:],
                        lhsT=encT[:, hc, :],
                        rhs=wclsT[:, hc, :],
                        start=(hc == 0),
                        stop=(hc == NCHUNK - 1),
                    )
                nc.scalar.activation(
                    out=lg[:, t, :],
                    in_=lgP[:],
                    func=mybir.ActivationFunctionType.Copy,
                )

            # ---- cono: nll = softplus((1-2y)*(l1-l0+dbc)) ----
            d01 = persist.tile([128, NT], f32)
            nc.vector.tensor_tensor(
                out=d01[:], in0=lg[:, :, 1], in1=lg[:, :, 0],
                op=mybir.AluOpType.subtract,
            )
            nc.vector.tensor_tensor(
                out=d01[:], in0=d01[:], in1=bcls[:, 1:2].to_broadcast([128, NT]),
                op=mybir.AluOpType.add,
            )
            nc.vector.tensor_tensor(
                out=d01[:], in0=d01[:], in1=bcls[:, 0:1].to_broadcast([128, NT]),
                op=mybir.AluOpType.subtract,
            )
            z = persist.tile([128, NT], f32)
            nc.vector.tensor_tensor(
                out=z[:], in0=d01[:], in1=sy[:], op=mybir.AluOpType.mult
            )
            # clamp to +-60 for Ln safety; softplus is linear/0 out there anyway
            nc.vector.tensor_scalar(
                out=z[:], in0=z[:], scalar1=60.0, scalar2=-60.0,
                op0=mybir.AluOpType.min, op1=mybir.AluOpType.max,
            )
            zg = persist.tile([128, NT], f32)
            nc.scalar.activation(
                out=zg[:], in_=z[:], func=mybir.ActivationFunctionType.Sigmoid,
                scale=-1.0,
            )
            zl = persist.tile([128, NT], f32)
            nc.scalar.activation(
                out=zl[:], in_=zg[:], func=mybir.ActivationFunctionType.Ln,
                accum_out=scono[:, 1:2],
            )

            # ---- phase 2: 11 context streams, processed as half-streams so
            # gather slots recycle against cvA before phase 1 fully finishes
            for k in range(NK):
                s = work.tile([128, NT], f32, tag="s")
                for h in range(2):
                    cv_half = cvA if h == 0 else cvB
                    g = gpool.tile([128, HNT, EMBED], f32, tag="g")
                    for t in range(HNT):
                        tt = h * HNT + t
                        nc.gpsimd.indirect_dma_start(
                            out=g[:, t, :],
                            out_offset=None,
                            in_=ctx_tab.ap(),
                            in_offset=bass.IndirectOffsetOnAxis(
                                ap=pids[:, (k + 1) * NT + tt : (k + 1) * NT + tt + 1],
                                axis=0,
                            ),
                        )
                    # in-place: overwrite the gathered rows with their products
                    nc.vector.tensor_tensor(
                        out=g[:], in0=g[:], in1=cv_half[:], op=mybir.AluOpType.mult
                    )
                    if k % 2 == 0:
                        nc.vector.tensor_reduce(
                            out=s[:, h * HNT : (h + 1) * HNT], in_=g[:],
                            axis=mybir.AxisListType.X, op=mybir.AluOpType.add,
                        )
                    else:
                        # balance: this stream's row-sums on ACT (free-dim accum)
                        for t in range(HNT):
                            tt = h * HNT + t
                            rtrash = work.tile([128, EMBED], f32, tag="rtrash")
                            nc.scalar.activation(
                                out=rtrash[:], in_=g[:, t, :],
                                func=mybir.ActivationFunctionType.Copy,
                                accum_out=s[:, tt : tt + 1],
                            )
                nc.vector.tensor_scalar(
                    out=s[:], in0=s[:], scalar1=10.0, scalar2=-10.0,
                    op0=mybir.AluOpType.min, op1=mybir.AluOpType.max,
                )
                sg = work.tile([128, NT], f32, tag="sg")
                # k=0 positive: softplus(-s) = -ln(sigmoid(s)); negs: softplus(s)
                nc.scalar.activation(
                    out=sg[:], in_=s[:],
                    func=mybir.ActivationFunctionType.Sigmoid,
                    scale=(1.0 if k == 0 else -1.0),
                )
                sl = work.tile([128, NT], f32, tag="sl")
                nc.scalar.activation(
                    out=sl[:], in_=sg[:],
                    func=mybir.ActivationFunctionType.Ln,
                    accum_out=deno_cols[:, k : k + 1],
                )

            # ---- final: sum columns, reduce partitions via PE ----
            nc.vector.tensor_reduce(
                out=scono[:, 0:1],
                in_=deno_cols[:],
                axis=mybir.AxisListType.X,
                op=mybir.AluOpType.add,
            )
            outP = psD.tile([1, 2], f32, tag="outP")
            nc.tensor.matmul(
                out=outP[:], lhsT=ones[:], rhs=scono[:], start=True, stop=True
            )
            out_sb = singles.tile([1, 2], f32)
            nc.vector.tensor_copy(out=out_sb[:], in_=outP[:])
            nc.sync.dma_start(out=out_d.ap(), in_=out_sb[:])

    nc.compile()
    return nc


def _get_program():
    global _PROGRAM
    if _PROGRAM is None:
        _PROGRAM = _build_program()
    return _PROGRAM


def _prep_inputs(center_word_ids, context_word_ids, neg_context_ids, party_label,
                 center_emb, context_emb, W_enc, b_enc, W_dec, b_dec, W_cls, b_cls):
    """Host-side staging: shard over cores, transpose/pad weights, cast ids."""
    center_word_ids = np.asarray(center_word_ids).astype(np.int32)
    context_word_ids = np.asarray(context_word_ids).astype(np.int32)
    neg_context_ids = np.asarray(neg_context_ids).astype(np.int32)
    party_label = np.asarray(party_label).astype(np.int32)
    center_emb = np.ascontiguousarray(np.asarray(center_emb, dtype=np.float32))
    context_emb = np.ascontiguousarray(np.asarray(context_emb, dtype=np.float32))
    W_enc = np.asarray(W_enc, dtype=np.float32)
    b_enc = np.asarray(b_enc, dtype=np.float32)
    W_dec = np.asarray(W_dec, dtype=np.float32)
    b_dec = np.asarray(b_dec, dtype=np.float32)
    W_cls = np.asarray(W_cls, dtype=np.float32)
    b_cls = np.asarray(b_cls, dtype=np.float32)

    # lhsT for enc: [e, h] = W_enc^T, padded to [384, 384], e-chunked on partitions
    wencT = np.zeros((PAD, PAD), np.float32)
    wencT[:EMBED, :HIDDEN] = W_enc.T
    wencT = np.ascontiguousarray(
        wencT.reshape(NCHUNK, 128, PAD).transpose(1, 0, 2)
    ).reshape(128, NCHUNK * PAD)

    # rhs for center_v: [h, e] = W_dec^T, h padded to 384, h-chunked
    wdecT = np.zeros((PAD, EMBED), np.float32)
    wdecT[:HIDDEN, :] = W_dec.T
    wdecT = np.ascontiguousarray(
        wdecT.reshape(NCHUNK, 128, EMBED).transpose(1, 0, 2)
    ).reshape(128, NCHUNK * EMBED)

    # rhs for logits: [h, 2] = W_cls^T
    wclsT = np.zeros((PAD, 2), np.float32)
    wclsT[:HIDDEN, :] = W_cls.T
    wclsT = np.ascontiguousarray(
        wclsT.reshape(NCHUNK, 128, 2).transpose(1, 0, 2)
    ).reshape(128, NCHUNK * 2)

    bencp = np.zeros((PAD,), np.float32)
    bencp[:HIDDEN] = b_enc
    bencp = np.ascontiguousarray(bencp.reshape(NCHUNK, 128).T)  # [128, 3]

    bdecp = b_dec.reshape(1, EMBED)
    bclsp = b_cls.reshape(1, 2)

    in_maps = []
    for c in range(N_CORES):
        sl = slice(c * BC, (c + 1) * BC)
        streams = np.empty((NK + 1, BC), np.int32)
        streams[0] = center_word_ids[sl]
        streams[1] = context_word_ids[sl]
        streams[2:] = neg_context_ids[sl].T
        # device layout: pids[p, k*NT+t] = streams[k, 128*t+p]
        pids = np.ascontiguousarray(
            streams.reshape(NK + 1, NT, 128).transpose(2, 0, 1).reshape(
                128, (NK + 1) * NT
            )
        )
        sy = np.ascontiguousarray(
            (1.0 - 2.0 * party_label[sl].astype(np.float32)).reshape(NT, 128).T
        )
        in_maps.append(
            {
                "center_tab": center_emb,
                "ctx_tab": context_emb,
                "pids": pids,
                "sy": sy,
                "wencT": wencT,
                "wdecT": wdecT,
                "wclsT": wclsT,
                "benc": bencp,
                "bdec": bdecp,
                "bcls": bclsp,
                "ident": np.eye(128, dtype=np.float32),
            }
        )
    return in_maps


def _postprocess(core_outs):
    # core_outs: [1,2] per core = (sum ln sigmoid(+-s), sum ln sigmoid(-z)); negate
    deno_sum = -sum(float(o[0, 0]) for o in core_outs)
    cono_sum = -sum(float(o[0, 1]) for o in core_outs)
    deno = np.clip(deno_sum / BATCH, EPS, 10.0)
    cono = np.clip(cono_sum / BATCH, EPS, 10.0)
    enc_loss = max(deno + cono, EPS)
    return np.array([enc_loss, deno, cono], dtype=np.float32)


def kernel(**inputs) -> np.ndarray:
    from concourse.bass_utils import run_bass_kernel_spmd

    nc = _get_program()
    in_maps = _prep_inputs(**inputs)
    res = run_bass_kernel_spmd(nc, in_maps, list(range(N_CORES)))
    return _postprocess([r["out"] for r in res.results])

